# revision 6
# baseline (speedup 1.0000x reference)
"""Trainium2 (Bass/Tile) kernel for BatchMarginRankingLoss over a PyG-style
batch of B=64 graphs x 1024 edges, SPMD on 8 NeuronCores (8 graphs/core).

Math
----
reference: for every graph, over all unordered slot pairs i<j:
    loss_ij = relu(sign(y_i - y_j) * (x_j - x_i)),
then per-graph mean over C = n(n-1)/2 pairs, then mean over graphs.

The full n x n pair-loss matrix L[p, f] = relu(sign(y_p - y_f) * (x_f - x_p))
is symmetric with zero diagonal, so sum_{i<j} L = 0.5 * sum_{p,f} L.
With w = x_f - x_p and H[p, f] = [y_f > y_p]:
    L[p, f] = relu(w) - H * w,
and summing the H*w term over a whole graph factorizes into matmuls:
    sum_{p,f} H*w = termA - termB,   termA = sum x_f * H,  termB = sum x_p * H.
Since H + H^T = 1 - I (up to measure-zero ties), termA = 1023*sum(x) - termB,
so only termB is needed:
    graph_total = sum relu(w) + 2*termB - 1023*sum(x).

Device mapping (per 128x1024 tile; 64 tiles/core/pass; raw w never built)
  VectorE : h2 = [Yrow > y_col]           (tensor_scalar is_gt bf16, 4x mode)
            rl = (Xrow - x_col) max 0     (fused two-op tensor_scalar,
                                           half the tiles)
  ScalarE : relu(Xrow + (-x_col)) + accum (activation Relu with per-partition
                                           bias, other half of the tiles)
  TensorE : psB += x_col^T @ h2           (termB, PSUM-accumulated, all tiles)
            psR += ones^T @ rl            (for the VectorE-relu tiles)
All inputs are broadcast-resident in SBUF (one [128, 8192] bf16 row tile per
tensor); per-partition scalar columns come from one PE transpose (identity
matrix passed as a host constant input).  Each core emits one f32 partial that
already includes the 1/(2*C*B) scaling; the host sums the 8 partials.
"""
import numpy as np
from contextlib import ExitStack

import concourse.bass as bass
import concourse.bacc as bacc
import concourse.tile as tile
from concourse import mybir
from concourse.alu_op_type import AluOpType
from concourse.bass import _add_dep_helper
from concourse.bass_utils import run_bass_kernel_spmd

B = 64            # graphs in the batch
PMAX = 1024       # edges per graph
N_CORES = 8
B_LOC = B // N_CORES            # 8 graphs per core
E_LOC = B_LOC * PMAX            # 8192 edges per core
CHUNKS = PMAX // 128            # 8 partition-chunks per graph
N_TILES = B_LOC * CHUNKS        # 64 tiles per core
PAIR_COUNT = PMAX * (PMAX - 1) // 2
SCALE = 1.0 / (2.0 * PAIR_COUNT * B)

F32 = mybir.dt.float32
BF16 = mybir.dt.bfloat16


def build_nc(reps: int = 1, loop_iters: int | None = None, variant: str = 'base') -> bacc.Bacc:
    """reps>1 unrolls the whole compute `reps` times; loop_iters=N wraps the
    main loop in a hardware For loop that runs it N times (same result; used
    to measure per-iteration HW time by wall-clock slope)."""
    nc = bacc.Bacc()
    x_ext = nc.declare_dram_parameter("x", [E_LOC], F32, isOutput=False)
    y_ext = nc.declare_dram_parameter("y", [E_LOC], F32, isOutput=False)
    out_ext = nc.declare_dram_parameter("out", [1, 1], F32, isOutput=True)

    with tile.TileContext(nc) as tc, ExitStack() as ctx:
        singles = ctx.enter_context(tc.tile_pool(name="singles", bufs=1))
        rows = ctx.enter_context(tc.tile_pool(name="rows", bufs=2))
        work = ctx.enter_context(tc.tile_pool(name="work", bufs=4))
        scratch = ctx.enter_context(tc.tile_pool(name="scratch", bufs=2))
        psum = ctx.enter_context(tc.tile_pool(name="psum", bufs=1, space="PSUM"))
        dram = ctx.enter_context(tc.tile_pool(name="dram", bufs=1, space="DRAM"))

        # ---- prologue: bf16 copies of x/y staged to DRAM scratch (source for
        # the per-graph broadcast-row DMAs)
        xbf_dram = dram.tile([B_LOC, PMAX], BF16)
        ybf_dram = dram.tile([B_LOC, PMAX], BF16)

        def stage_bf16(ext, bf_dram, tag):
            g8_f = singles.tile([B_LOC, PMAX], F32, tag=f"{tag}_g8f")
            nc.sync.dma_start(g8_f[:], ext[:].rearrange("(g n) -> g n", g=B_LOC))
            g8 = singles.tile([B_LOC, PMAX], BF16, tag=f"{tag}_g8")
            nc.vector.tensor_copy(g8[:], g8_f[:])
            nc.sync.dma_start(bf_dram[:], g8[:])
            return g8_f

        xg8_f_tile = yg8_f_tile = None
        if variant != "empty":
            xg8_f_tile = stage_bf16(x_ext, xbf_dram, "x")
            yg8_f_tile = stage_bf16(y_ext, ybf_dram, "y")

        # per-partition scalar columns, one [128, CHUNKS] f32 tile per graph:
        # xcol_g[p, r] = x[g*PMAX + 128*r + p]  (strided 4KB DMA from DRAM)
        xcols, ycols, xcols_bf = [], [], []
        for g in range(B_LOC if variant != "empty" else 0):
            xc = singles.tile([128, CHUNKS], F32, tag=f"xcol{g}")
            nc.sync.dma_start(
                xc[:], x_ext[g * PMAX:(g + 1) * PMAX].rearrange("(r p) -> p r", p=128))
            yc = singles.tile([128, CHUNKS], F32, tag=f"ycol{g}")
            nc.sync.dma_start(
                yc[:], y_ext[g * PMAX:(g + 1) * PMAX].rearrange("(r p) -> p r", p=128))
            xcols.append(xc)
            ycols.append(yc)
            if variant.startswith("mmB"):
                xcb = singles.tile([128, CHUNKS], BF16, tag=f"xcolbf{g}")
                nc.vector.tensor_copy(xcb[:], xc[:])
                xcols_bf.append(xcb)

        rcols = singles.tile([128, N_TILES], F32)
        if variant.startswith("mmB"):
            D_all = singles.tile([B_LOC, PMAX], F32)
            psB = psum.tile([1, PMAX], F32, tag="psB")
            ones8 = singles.tile([B_LOC, 1], F32)
            nc.vector.memset(ones8[:], 1.0)
            ones1 = singles.tile([1, 1], F32)
            nc.vector.memset(ones1[:], 1.0)
        if variant in ("norelu", "nott", "empty"):
            nc.vector.memset(rcols[:], 0.0)
        ones_bf = singles.tile([128, 1], BF16)
        nc.vector.memset(ones_bf[:], 1.0)
        # PSUM accumulator for sum_p of all gs tiles: [1, PMAX] f32
        if not variant.startswith("mmB"):
            psA = psum.tile([1, PMAX], F32)
        if variant in ("nott", "empty"):
            nc.vector.memset(psA[:], 0.0)

        # resident broadcast rows: all 8 graphs' X/Y rows live in SBUF
        Xrows, Yrows = [], []
        if variant not in ("dma_rows", "empty"):
            engs = [nc.sync, nc.scalar, nc.gpsimd]
            for g in range(B_LOC):
                Xr = singles.tile([128, PMAX], BF16, tag=f"Xrow{g}")
                engs[(2 * g) % len(engs)].dma_start(
                    Xr[:], xbf_dram[g:g + 1, :].partition_broadcast(128))
                Yr = singles.tile([128, PMAX], BF16, tag=f"Yrow{g}")
                engs[(2 * g + 1) % len(engs)].dma_start(
                    Yr[:], ybf_dram[g:g + 1, :].partition_broadcast(128))
                Xrows.append(Xr)
                Yrows.append(Yr)

        # ---- main loop: 8 graphs x 8 chunks (x reps)
        import contextlib
        loop_cm = (tc.For_i(0, loop_iters, 1) if loop_iters
                   else contextlib.nullcontext())
        with loop_cm:
            if variant == "empty":
                etile = work.tile([128, 1], F32, tag="etile")
                nc.vector.memset(etile[:], 0.0)
            for rep in range(reps):
                if variant == "empty":
                    break
                for g in range(B_LOC):
                    if variant == "dma_rows":
                        Xrow = rows.tile([128, PMAX], BF16, tag="Xrow")
                        nc.sync.dma_start(
                            Xrow[:], xbf_dram[g:g + 1, :].partition_broadcast(128))
                        Yrow = rows.tile([128, PMAX], BF16, tag="Yrow")
                        nc.sync.dma_start(
                            Yrow[:], ybf_dram[g:g + 1, :].partition_broadcast(128))
                    else:
                        Xrow, Yrow = Xrows[g], Yrows[g]
                    if variant.startswith("mmB"):
                        psD = psum.tile([1, PMAX], F32, tag="psD")
                    for r in range(CHUNKS):
                        t = g * CHUNKS + r
                        w = work.tile([128, PMAX], BF16, tag="w")
                        nc.vector.tensor_scalar(
                            w[:], Xrow[:], xcols[g][:, r:r + 1], None,
                            AluOpType.subtract)
                        h2 = work.tile([128, PMAX], BF16, tag="h2")
                        nc.vector.tensor_scalar(
                            h2[:], Yrow[:], ycols[g][:, r:r + 1], None,
                            AluOpType.is_gt)
                        if variant.startswith("mmB"):
                            # term B: sum_p x_p * H  (accumulate over ALL tiles)
                            # term A prep: D_g[f] = sum_p H[p, f]  (per graph)
                            for half in range(2):
                                sl = slice(half * 512, (half + 1) * 512)
                                nc.tensor.matmul(
                                    psB[:, sl], xcols_bf[g][:, r:r + 1], h2[:, sl],
                                    start=(t == 0), stop=(t == N_TILES - 1))
                                nc.tensor.matmul(
                                    psD[:, sl], ones_bf[:], h2[:, sl],
                                    start=(r == 0), stop=(r == CHUNKS - 1))
                        elif variant != "nott":
                            gs = scratch.tile([128, PMAX], BF16, tag="gs")
                            tt_eng = (nc.gpsimd if (variant == "ttg" and t % 2 == 0)
                                      else nc.vector)
                            tt_eng.tensor_tensor(gs[:], h2[:], w[:],
                                                 AluOpType.mult)
                            for half in range(2):
                                nc.tensor.matmul(
                                    psA[:, half * 512:(half + 1) * 512],
                                    ones_bf[:],
                                    gs[:, half * 512:(half + 1) * 512],
                                    start=(t == 0), stop=(t == N_TILES - 1))
                        if variant != "norelu":
                            rs = scratch.tile([128, PMAX], BF16, tag="rs")
                            if variant == "relu_v":
                                nc.vector.tensor_scalar(
                                    rs[:], w[:], 0.0, 0.0, AluOpType.max,
                                    AluOpType.add,
                                    accum_out=rcols[:, t:t + 1])
                            elif variant == "relu_g":
                                nc.gpsimd.tensor_scalar(
                                    rs[:], w[:], 0.0, 0.0, AluOpType.max,
                                    AluOpType.add,
                                    accum_out=rcols[:, t:t + 1])
                            elif variant == "relu_mix":
                                eng = nc.gpsimd if (t % 2 == 0) else nc.scalar
                                if eng is nc.scalar:
                                    nc.scalar.activation(
                                        rs[:], w[:],
                                        mybir.ActivationFunctionType.Relu,
                                        accum_out=rcols[:, t:t + 1])
                                else:
                                    nc.gpsimd.tensor_scalar(
                                        rs[:], w[:], 0.0, 0.0, AluOpType.max,
                                        AluOpType.add,
                                        accum_out=rcols[:, t:t + 1])
                            else:
                                nc.scalar.activation(
                                    rs[:], w[:],
                                    mybir.ActivationFunctionType.Relu,
                                    accum_out=rcols[:, t:t + 1])
                    if variant.startswith("mmB"):
                        nc.vector.tensor_copy(D_all[g:g + 1, :], psD[:])

        if variant.startswith("mmB"):
            # total = sum(rcols) + sum(psB) - sum_g dot(x_g, D_g), all * SCALE
            dsum = singles.tile([128, 1], F32)
            nc.vector.tensor_reduce(dsum[:], rcols[:], mybir.AxisListType.X,
                                    AluOpType.add)
            prod = singles.tile([B_LOC, PMAX], F32)
            nc.vector.tensor_tensor(prod[:], D_all[:], xg8_f_tile[:],
                                    AluOpType.mult)
            prodsum = singles.tile([B_LOC, 1], F32)
            nc.vector.tensor_reduce(prodsum[:], prod[:], mybir.AxisListType.X,
                                    AluOpType.add)
            prodneg = singles.tile([B_LOC, 1], F32)
            nc.vector.tensor_scalar(prodneg[:], prodsum[:], -1.0, None,
                                    AluOpType.mult)
            psBsum = singles.tile([1, 1], F32)
            nc.vector.tensor_reduce(psBsum[:], psB[:], mybir.AxisListType.X,
                                    AluOpType.add)
            ones = singles.tile([128, 1], F32)
            nc.vector.memset(ones[:], 1.0)
            ps = psum.tile([1, 1], F32)
            nc.tensor.matmul(ps[:], ones[:], dsum[:], start=True, stop=False)
            nc.tensor.matmul(ps[:], ones8[:], prodneg[:], start=False, stop=False)
            nc.tensor.matmul(ps[:], ones1[:], psBsum[:], start=False, stop=True)
            outsb = singles.tile([1, 1], F32)
            nc.scalar.activation(outsb[:], ps[:],
                                 mybir.ActivationFunctionType.Identity,
                                 scale=float(SCALE))
            nc.sync.dma_start(out_ext[:], outsb[:])
        else:
            # ---- epilogue: total = (sum(rcols) - sum(psA)) * SCALE
            dsum = singles.tile([128, 1], F32)
            nc.vector.tensor_reduce(dsum[:], rcols[:], mybir.AxisListType.X,
                                    AluOpType.add)
            ones = singles.tile([128, 1], F32)
            nc.vector.memset(ones[:], 1.0)
            ps = psum.tile([1, 1], F32)
            nc.tensor.matmul(ps[:], ones[:], dsum[:], start=True, stop=True)
            gtot = singles.tile([1, 1], F32)
            nc.vector.tensor_reduce(gtot[:], psA[:], mybir.AxisListType.X,
                                    AluOpType.add)
            rtot = singles.tile([1, 1], F32)
            nc.scalar.activation(rtot[:], ps[:],
                                 mybir.ActivationFunctionType.Identity)
            diff = singles.tile([1, 1], F32)
            nc.vector.tensor_tensor(diff[:], rtot[:], gtot[:], AluOpType.subtract)
            outsb = singles.tile([1, 1], F32)
            nc.scalar.activation(outsb[:], diff[:],
                                 mybir.ActivationFunctionType.Identity,
                                 scale=float(SCALE))
            nc.sync.dma_start(out_ext[:], outsb[:])

    nc.finalize()
    return nc


def build_nc2(reps: int = 1, loop_iters: int | None = None,
              accum_mod: int = 3, accum_keep: int = 2) -> bacc.Bacc:
    """Balanced-engine build: per tile
         V:  w = Xrow - x_col; h2 = [Yrow > y_col]; h2t = [Yrow < y_col]
         PE: psA2 += xcol_bf @ h2t ; psB += xcol_bf @ h2   (both Sum H*w terms)
         ACT: relu(w) with accum (accum_keep of accum_mod tiles) or plain relu
              + PE ones-matmul reduction for the rest
       total = sum(rcols) + sum(psR) + sum(psB) - sum(psA2), * SCALE.
    """
    nc = bacc.Bacc()
    x_ext = nc.declare_dram_parameter("x", [E_LOC], F32, isOutput=False)
    y_ext = nc.declare_dram_parameter("y", [E_LOC], F32, isOutput=False)
    ident_ext = nc.declare_dram_parameter("ident", [64, 64], F32, isOutput=False)
    out_ext = nc.declare_dram_parameter("out", [1, 1], F32, isOutput=True)

    with tile.TileContext(nc) as tc, ExitStack() as ctx:
        singles = ctx.enter_context(tc.tile_pool(name="singles", bufs=1))
        work = ctx.enter_context(tc.tile_pool(name="work", bufs=4))
        scratch = ctx.enter_context(tc.tile_pool(name="scratch", bufs=3))
        psum = ctx.enter_context(tc.tile_pool(name="psum", bufs=1, space="PSUM"))
        dram = ctx.enter_context(tc.tile_pool(name="dram", bufs=1, space="DRAM"))

        xbf_dram = dram.tile([B_LOC, PMAX], BF16)
        ybf_dram = dram.tile([B_LOC, PMAX], BF16)

        def stage_bf16(ext, bf_dram, tag):
            g8_f = singles.tile([B_LOC, PMAX], F32, tag=f"{tag}_g8f")
            nc.sync.dma_start(g8_f[:], ext[:].rearrange("(g n) -> g n", g=B_LOC))
            g8 = singles.tile([B_LOC, PMAX], BF16, tag=f"{tag}_g8")
            nc.vector.tensor_copy(g8[:], g8_f[:])
            nc.sync.dma_start(bf_dram[:], g8[:])
            return g8_f

        # per-partition scalar columns via PE transpose:
        # xin64 [64, 128] (straight) -> xcol_all [128, 64] with
        # xcol_all[p, t] = x[128 t + p]
        ident_sb = singles.tile([64, 64], F32)
        nc.sync.dma_start(ident_sb[:], ident_ext[:])
        xcol_all = singles.tile([128, 64], F32)
        ycol_all = singles.tile([128, 64], F32)
        xcol_all_bf = singles.tile([128, 64], BF16)
        for ext, dst, dst_bf, eng in ((x_ext, xcol_all, xcol_all_bf, nc.scalar),
                                      (y_ext, ycol_all, None, nc.gpsimd)):
            in64 = work.tile([64, 128], F32, tag="in64")
            eng.dma_start(in64[:], ext[:].rearrange("(c p) -> c p", p=128))
            psT = psum.tile([128, 64], F32, tag="psT")
            nc.tensor.matmul(psT[:], in64[:], ident_sb[:], is_transpose=True,
                             start=True, stop=True)
            nc.vector.tensor_copy(dst[:], psT[:])
            if dst_bf is not None:
                nc.vector.tensor_copy(dst_bf[:], psT[:])
        negxcol_all = singles.tile([128, 64], F32)
        nc.vector.tensor_scalar(negxcol_all[:], xcol_all[:], -1.0, None,
                                AluOpType.mult)

        xg8_f = stage_bf16(x_ext, xbf_dram, "x")
        stage_bf16(y_ext, ybf_dram, "y")

        # resident broadcast rows: one [128, E_LOC] tile per tensor, loaded
        # by 2 half DMAs each (128 contiguous-run descriptors per DMA)
        Xall = singles.tile([128, E_LOC], BF16)
        Yall = singles.tile([128, E_LOC], BF16)
        # graph-0 pieces first (small, unblock compute), then two big pieces
        pieces = [(0, 2 * PMAX), (2 * PMAX, E_LOC)]
        eng_rr = [nc.sync, nc.scalar, nc.gpsimd]
        k = 0
        for lo, hi in pieces:
            for src, dst in ((ybf_dram, Yall), (xbf_dram, Xall)):
                flat = src[:].rearrange("g n -> (g n)")
                eng_rr[k % 3].dma_start(
                    dst[:, lo:hi],
                    flat[lo:hi].unsqueeze(0).partition_broadcast(128))
                k += 1
        Xrows = [Xall[:, g * PMAX:(g + 1) * PMAX] for g in range(B_LOC)]
        Yrows = [Yall[:, g * PMAX:(g + 1) * PMAX] for g in range(B_LOC)]

        rcols = singles.tile([128, N_TILES], F32)
        nc.vector.memset(rcols[:], 0.0)
        ones_bf = singles.tile([128, 1], BF16)
        nc.vector.memset(ones_bf[:], 1.0)
        psB = psum.tile([1, PMAX], F32, tag="psB")
        psR = psum.tile([1, PMAX], F32, tag="psR")

        import contextlib
        loop_cm = (tc.For_i(0, loop_iters, 1) if loop_iters
                   else contextlib.nullcontext())
        n_acc = 0
        n_mm = 0
        mm_ts = [t for t in range(N_TILES) if t % accum_mod < accum_keep]
        last_mm_t = mm_ts[-1] if mm_ts else None
        with loop_cm:
            for rep in range(reps):
                first = (rep == 0)
                last = (rep == reps - 1)
                for g in range(B_LOC):
                    Xrow, Yrow = Xrows[g], Yrows[g]
                    for r in range(CHUNKS):
                        t = g * CHUNKS + r
                        c = 8 * g + r
                        h2 = work.tile([128, PMAX], BF16, tag="h2")
                        nc.vector.tensor_scalar(
                            h2[:], Yrow[:], ycol_all[:, c:c + 1],
                            None, AluOpType.is_gt)
                        for half in range(2):
                            sl = slice(half * 512, (half + 1) * 512)
                            nc.tensor.matmul(
                                psB[:, sl], xcol_all_bf[:, c:c + 1], h2[:, sl],
                                start=(first and t == 0),
                                stop=(last and t == N_TILES - 1))
                        rl = scratch.tile([128, PMAX], BF16, tag="rl")
                        if t % accum_mod < accum_keep:
                            # rl = relu(Xrow - x_col) in one fused DVE op
                            nc.vector.tensor_scalar(
                                rl[:], Xrow[:], xcol_all[:, c:c + 1], 0.0,
                                AluOpType.subtract, AluOpType.max)
                            for half in range(2):
                                sl = slice(half * 512, (half + 1) * 512)
                                nc.tensor.matmul(
                                    psR[:, sl], ones_bf[:], rl[:, sl],
                                    start=(first and t == mm_ts[0]),
                                    stop=(last and t == last_mm_t))
                            n_mm += 1
                        else:
                            # relu(Xrow + (-x_col)) + accum directly on ScalarE
                            nc.scalar.activation(
                                rl[:], Xrow[:],
                                mybir.ActivationFunctionType.Relu,
                                bias=negxcol_all[:, c:c + 1], scale=1.0,
                                accum_out=rcols[:, t:t + 1])


        # epilogue: total = sum(rcols) + sum(psR) + 2*sum(psB) - 1023*sum(x)
        dsum = singles.tile([128, 1], F32)
        nc.vector.tensor_reduce(dsum[:], rcols[:], mybir.AxisListType.X,
                                AluOpType.add)
        ones128e = singles.tile([128, 1], F32)
        nc.vector.memset(ones128e[:], 1.0)
        psum_r = singles.tile([1, 1], F32)
        nc.vector.tensor_reduce(psum_r[:], psR[:], mybir.AxisListType.X,
                                AluOpType.add)
        psum_b = singles.tile([1, 1], F32)
        dummy_b = singles.tile([1, PMAX], F32)
        nc.scalar.activation(dummy_b[:], psB[:],
                             mybir.ActivationFunctionType.Identity,
                             accum_out=psum_b[:])
        xsum8 = singles.tile([B_LOC, 1], F32)
        nc.vector.tensor_reduce(xsum8[:], xg8_f[:], mybir.AxisListType.X,
                                AluOpType.add)
        xsum8n = singles.tile([B_LOC, 1], F32)
        nc.vector.tensor_scalar(xsum8n[:], xsum8[:], -float(PMAX - 1), None,
                                AluOpType.mult)
        c1 = singles.tile([1, 1], F32)
        nc.vector.tensor_scalar(c1[:], psum_b[:], 2.0, None, AluOpType.mult)
        c2 = singles.tile([1, 1], F32)
        nc.vector.tensor_tensor(c2[:], c1[:], psum_r[:], AluOpType.add)
        ones8e = singles.tile([B_LOC, 1], F32)
        nc.vector.memset(ones8e[:], 1.0)
        ones1 = singles.tile([1, 1], F32)
        nc.vector.memset(ones1[:], 1.0)
        ps = psum.tile([1, 1], F32, tag="psfin")
        nc.tensor.matmul(ps[:], ones128e[:], dsum[:], start=True, stop=False)
        nc.tensor.matmul(ps[:], ones8e[:], xsum8n[:], start=False, stop=False)
        nc.tensor.matmul(ps[:], ones1[:], c2[:], start=False, stop=True)
        outsb = singles.tile([1, 1], F32)
        nc.scalar.activation(outsb[:], ps[:],
                             mybir.ActivationFunctionType.Identity,
                             scale=float(SCALE))
        nc.sync.dma_start(out_ext[:], outsb[:])

    nc.finalize()
    return nc


def build_nc3(reps: int = 1, loop_iters: int | None = None,
              n_act: int = 12, n_gps: int = 9, act0: int = 26) -> bacc.Bacc:
    """Rank-identity build: the whole loss collapses to
         total = sum_g sum_i x_i * (rank_x(i) - rank_y(i)) / (C*B),
       and ranks are row-sums of comparison tiles, obtained for free via
       accum_out.  Main loop = 128 elementwise ops (64 tiles x {x,y}), NO
       matmuls / PSUM at all:
         DVE : out = is_gt(Row, col)  (tensor_scalar fp16, 4x mode, accum)
         ACT : out = Sign(Row - col)  (activation, accum)  [affine of rank]
         GPS : out = is_gt(Row, col)  (tensor_scalar, accum)
       Rows are fp16; compare scalars are fp16-rounded (so the self-compare
       is exactly 0).  Epilogue: D = (raccY - raccX) * wAx, reduce, scale.
       Tiles [act0, act0+n_act) go to ScalarE (Sign), the last n_gps to
       GpSimd, the rest to DVE."""
    nc = bacc.Bacc()
    x_ext = nc.declare_dram_parameter("x", [E_LOC], F32, isOutput=False)
    y_ext = nc.declare_dram_parameter("y", [E_LOC], F32, isOutput=False)
    ident_ext = nc.declare_dram_parameter("ident", [64, 64], F32, isOutput=False)
    out_ext = nc.declare_dram_parameter("out", [1, 1], F32, isOutput=True)

    FP16 = mybir.dt.float16
    act_ts = set(range(act0, act0 + n_act))
    gps_ts = set(range(N_TILES - n_gps, N_TILES))

    with tile.TileContext(nc) as tc, ExitStack() as ctx:
        singles = ctx.enter_context(tc.tile_pool(name="singles", bufs=1))
        work = ctx.enter_context(tc.tile_pool(name="work", bufs=8))
        ascr = ctx.enter_context(tc.tile_pool(name="ascr", bufs=4))
        gscr = ctx.enter_context(tc.tile_pool(name="gscr", bufs=4))
        psum = ctx.enter_context(tc.tile_pool(name="psum", bufs=1, space="PSUM"))
        dram = ctx.enter_context(tc.tile_pool(name="dram", bufs=1, space="DRAM"))

        # ---- per-partition scalar columns via PE transpose (as build_nc2)
        ident_sb = singles.tile([64, 64], F32)
        nc.sync.dma_start(ident_sb[:], ident_ext[:])
        xcol_all = singles.tile([128, 64], F32)   # exact f32 (epilogue weight)
        xcol_r = singles.tile([128, 64], F32)     # fp16-rounded compare scalar
        ycol_r = singles.tile([128, 64], F32)
        negx_r = singles.tile([128, 64], F32)     # -rounded (ACT Sign bias)
        negy_r = singles.tile([128, 64], F32)
        col16 = singles.tile([128, 64], FP16)
        for ext, dst_f32, dst_r, dst_neg, eng in (
                (x_ext, xcol_all, xcol_r, negx_r, nc.scalar),
                (y_ext, None, ycol_r, negy_r, nc.gpsimd)):
            in64 = work.tile([64, 128], F32, tag="in64")
            eng.dma_start(in64[:], ext[:].rearrange("(c p) -> c p", p=128))
            psT = psum.tile([128, 64], F32, tag="psT")
            nc.tensor.matmul(psT[:], in64[:], ident_sb[:], is_transpose=True,
                             start=True, stop=True)
            if dst_f32 is not None:
                nc.vector.tensor_copy(dst_f32[:], psT[:])
            nc.vector.tensor_copy(col16[:], psT[:])       # round to fp16
            nc.vector.tensor_copy(dst_r[:], col16[:])     # back to exact f32
            nc.vector.tensor_scalar(dst_neg[:], dst_r[:], -1.0, None,
                                    AluOpType.mult)

        # ---- fp16 broadcast rows (stage fp16 to DRAM, then bcast-DMA)
        x16_dram = dram.tile([B_LOC, PMAX], FP16)
        y16_dram = dram.tile([B_LOC, PMAX], FP16)
        for ext, d16, tag in ((x_ext, x16_dram, "x"), (y_ext, y16_dram, "y")):
            g8_f = work.tile([B_LOC, PMAX], F32, tag=f"{tag}_g8f")
            nc.sync.dma_start(g8_f[:], ext[:].rearrange("(g n) -> g n", g=B_LOC))
            g8 = work.tile([B_LOC, PMAX], FP16, tag=f"{tag}_g8")
            nc.vector.tensor_copy(g8[:], g8_f[:])
            nc.sync.dma_start(d16[:], g8[:])
        Xall = singles.tile([128, E_LOC], FP16)
        Yall = singles.tile([128, E_LOC], FP16)
        pieces = [(0, 2 * PMAX), (2 * PMAX, E_LOC)]
        eng_rr = [nc.sync, nc.scalar, nc.gpsimd]
        k = 0
        for lo, hi in pieces:
            for src, dst in ((y16_dram, Yall), (x16_dram, Xall)):
                flat = src[:].rearrange("g n -> (g n)")
                eng_rr[k % 3].dma_start(
                    dst[:, lo:hi],
                    flat[lo:hi].unsqueeze(0).partition_broadcast(128))
                k += 1
        Xrows = [Xall[:, g * PMAX:(g + 1) * PMAX] for g in range(B_LOC)]
        Yrows = [Yall[:, g * PMAX:(g + 1) * PMAX] for g in range(B_LOC)]

        # epilogue weight: wAx[p,t] = x_exact * (+1 | +0.5 for ACT tiles)
        wAx = singles.tile([128, 64], F32)
        nc.vector.tensor_copy(wAx[:], xcol_all[:])
        if n_act:
            nc.vector.tensor_scalar(wAx[:, act0:act0 + n_act],
                                    xcol_all[:, act0:act0 + n_act],
                                    0.5, None, AluOpType.mult)

        raccX = singles.tile([128, N_TILES], F32)
        raccY = singles.tile([128, N_TILES], F32)

        import contextlib
        loop_cm = (tc.For_i(0, loop_iters, 1) if loop_iters
                   else contextlib.nullcontext())
        with loop_cm:
            for rep in range(reps):
                for t in range(N_TILES):
                    g, a = divmod(t, CHUNKS)
                    Xrow, Yrow = Xrows[g], Yrows[g]
                    for (Row, colr, negr, racc) in (
                            (Xrow, xcol_r, negx_r, raccX),
                            (Yrow, ycol_r, negy_r, raccY)):
                        if t in act_ts:
                            scr = ascr.tile([128, PMAX], FP16, tag="ascr")
                            nc.scalar.activation(
                                scr[:], Row[:],
                                mybir.ActivationFunctionType.Sign,
                                bias=negr[:, t:t + 1], scale=1.0,
                                accum_out=racc[:, t:t + 1])
                        elif t in gps_ts:
                            scr = gscr.tile([128, PMAX], FP16, tag="gscr")
                            nc.gpsimd.tensor_scalar(
                                scr[:], Row[:], colr[:, t:t + 1], 0.0,
                                AluOpType.is_gt, AluOpType.add,
                                accum_out=racc[:, t:t + 1])
                        else:
                            scr = work.tile([128, PMAX], FP16, tag="wscr")
                            nc.vector.tensor_scalar(
                                scr[:], Row[:], colr[:, t:t + 1], 0.0,
                                AluOpType.is_gt, AluOpType.add,
                                accum_out=racc[:, t:t + 1])

        # ---- epilogue: total = sum_p sum_t wAx * (raccY - raccX), * SCALE3
        D = singles.tile([128, N_TILES], F32)
        nc.vector.tensor_tensor(D[:], raccY[:], raccX[:], AluOpType.subtract)
        P = singles.tile([128, N_TILES], F32)
        nc.vector.tensor_tensor(P[:], D[:], wAx[:], AluOpType.mult)
        dsum = singles.tile([128, 1], F32)
        nc.vector.tensor_reduce(dsum[:], P[:], mybir.AxisListType.X,
                                AluOpType.add)
        ones128e = singles.tile([128, 1], F32)
        nc.vector.memset(ones128e[:], 1.0)
        ps = psum.tile([1, 1], F32, tag="psfin")
        nc.tensor.matmul(ps[:], ones128e[:], dsum[:], start=True, stop=True)
        outsb = singles.tile([1, 1], F32)
        nc.scalar.activation(outsb[:], ps[:],
                             mybir.ActivationFunctionType.Identity,
                             scale=float(1.0 / (PAIR_COUNT * B)))
        nc.sync.dma_start(out_ext[:], outsb[:])

    nc.finalize()
    return nc


class _Runner:
    """Persistent compiled executor for the SPMD bass program: traces and
    compiles the jit once, then each call is just a dispatch. Mirrors
    concourse.bass2jax.run_bass_via_pjrt's multi-core branch."""

    def __init__(self, nc):
        import jax
        from jax.experimental.shard_map import shard_map
        from jax.sharding import Mesh, PartitionSpec
        from concourse import bass2jax

        bass2jax.install_neuronx_cc_hook()
        self.nc = nc
        in_names, out_names, out_avals, zero_outs = [], [], [], []
        partition_name = (nc.partition_id_tensor.name
                          if nc.partition_id_tensor else None)
        for alloc in nc.m.functions[0].allocations:
            if not isinstance(alloc, mybir.MemoryLocationSet):
                continue
            name = alloc.memorylocations[0].name
            if alloc.kind == "ExternalInput":
                if name != partition_name:
                    in_names.append(name)
            elif alloc.kind == "ExternalOutput":
                shape = tuple(alloc.tensor_shape)
                dtype = mybir.dt.np(alloc.dtype)
                out_names.append(name)
                out_avals.append(jax.core.ShapedArray(shape, dtype))
                zero_outs.append(np.zeros(shape, dtype))
        n_params = len(in_names)
        n_outs = len(out_avals)
        all_in_names = list(in_names) + list(out_names)
        if partition_name is not None:
            all_in_names.append(partition_name)
        self.in_names = in_names
        self.out_names = out_names
        self.zero_outs = zero_outs
        donate = tuple(range(n_params, n_params + n_outs))

        def _body(*args):
            operands = list(args)
            if partition_name is not None:
                operands.append(bass2jax.partition_id_tensor())
            outs = bass2jax._bass_exec_p.bind(
                *operands,
                out_avals=tuple(out_avals),
                in_names=tuple(all_in_names),
                out_names=tuple(out_names),
                lowering_input_output_aliases=(),
                sim_require_finite=True,
                sim_require_nnan=True,
                nc=nc,
            )
            return tuple(outs)

        devices = jax.devices()[:N_CORES]
        assert len(devices) == N_CORES
        mesh = Mesh(np.asarray(devices), ("core",))
        in_specs = (PartitionSpec("core"),) * (n_params + n_outs)
        out_specs = (PartitionSpec("core"),) * n_outs
        self._jit = jax.jit(
            shard_map(_body, mesh=mesh, in_specs=in_specs, out_specs=out_specs,
                      check_rep=False),
            donate_argnums=donate, keep_unused=True)

    def __call__(self, in_maps):
        import jax
        if "ident" in self.in_names and "ident" not in in_maps[0]:
            eye = np.eye(64, dtype=np.float32)
            in_maps = [{**m, "ident": eye} for m in in_maps]
        concat_in = [
            np.concatenate([np.asarray(in_maps[c][k]) for c in range(N_CORES)],
                           axis=0)
            for k in self.in_names
        ]
        zeros = [np.concatenate([z] * N_CORES, axis=0) for z in self.zero_outs]
        outs = self._jit(*concat_in, *zeros)
        outs = [np.asarray(o) for o in jax.block_until_ready(outs)]
        res = []
        for c in range(N_CORES):
            m = {}
            for i, name in enumerate(self.out_names):
                n0 = self.zero_outs[i].shape[0]
                m[name] = outs[i][c * n0:(c + 1) * n0]
            res.append(m)
        return res


_RUNNERS: dict = {}


def get_runner(reps: int = 1, loop_iters: int | None = None,
               variant: str = "base") -> _Runner:
    key = (reps, loop_iters, variant)
    if key not in _RUNNERS:
        if variant.startswith("r3"):
            parts = variant.split("_")
            na = int(parts[1]) if len(parts) > 1 else 12
            ng = int(parts[2]) if len(parts) > 2 else 9
            a0 = int(parts[3]) if len(parts) > 3 else 26
            _RUNNERS[key] = _Runner(build_nc3(reps, loop_iters, na, ng, a0))
        elif variant.startswith("v2"):
            parts = variant.split("_")
            am = int(parts[1]) if len(parts) > 2 else 3
            ak = int(parts[2]) if len(parts) > 2 else 2
            _RUNNERS[key] = _Runner(build_nc2(reps, loop_iters, am, ak))
        else:
            _RUNNERS[key] = _Runner(build_nc(reps, loop_iters, variant))
    return _RUNNERS[key]


def kernel(outputs: np.ndarray, y: np.ndarray, edges_batch: np.ndarray) -> np.ndarray:
    outputs = np.ascontiguousarray(np.asarray(outputs, dtype=np.float32))
    y = np.ascontiguousarray(np.asarray(y, dtype=np.float32))
    eb = np.asarray(edges_batch)
    assert outputs.shape == (B * PMAX,) and y.shape == (B * PMAX,)
    # this kernel is specialized to the PyG-style equal-sized-graph batch the
    # problem generates: edges_batch == repeat(arange(B), PMAX)
    expected_eb = np.repeat(np.arange(B, dtype=eb.dtype), PMAX)
    assert np.array_equal(eb, expected_eb), "kernel requires equal-sized graphs"

    in_maps = [
        {"x": outputs[i * E_LOC:(i + 1) * E_LOC], "y": y[i * E_LOC:(i + 1) * E_LOC]}
        for i in range(N_CORES)
    ]
    res = get_runner(1, variant="v2_2_1")(in_maps)
    total = np.float64(0.0)
    for i in range(N_CORES):
        total += np.float64(res[i]["out"][0, 0])
    return np.asarray(total, dtype=np.float32)



# revision 11
# speedup vs baseline: 1.1217x; 1.1217x over previous
"""Trainium2 (Bass/Tile) kernel for BatchMarginRankingLoss over a PyG-style
batch of B=64 graphs x 1024 edges, SPMD on 8 NeuronCores (8 graphs/core).

Math
----
reference: for every graph, over all unordered slot pairs i<j:
    loss_ij = relu(sign(y_i - y_j) * (x_j - x_i)),
then per-graph mean over C = n(n-1)/2 pairs, then mean over graphs.

The full n x n pair-loss matrix L[p, f] = relu(sign(y_p - y_f) * (x_f - x_p))
is symmetric with zero diagonal, so sum_{i<j} L = 0.5 * sum_{p,f} L.
With w = x_f - x_p and H[p, f] = [y_f > y_p]:
    L[p, f] = relu(w) - H * w,
and summing the H*w term over a whole graph factorizes into matmuls:
    sum_{p,f} H*w = termA - termB,   termA = sum x_f * H,  termB = sum x_p * H.
Since H + H^T = 1 - I (up to measure-zero ties), termA = 1023*sum(x) - termB,
so only termB is needed:
    graph_total = sum relu(w) + 2*termB - 1023*sum(x).

Device mapping (per 128x1024 tile; 64 tiles/core/pass; raw w never built)
  VectorE : h2 = [Yrow > y_col]           (tensor_scalar is_gt bf16, 4x mode)
            rl = (Xrow - x_col) max 0     (fused two-op tensor_scalar,
                                           half the tiles)
  ScalarE : relu(Xrow + (-x_col)) + accum (activation Relu with per-partition
                                           bias, other half of the tiles)
  TensorE : psB += x_col^T @ h2           (termB, PSUM-accumulated, all tiles)
            psR += ones^T @ rl            (for the VectorE-relu tiles)
All inputs are broadcast-resident in SBUF (one [128, 8192] bf16 row tile per
tensor); per-partition scalar columns come from one PE transpose (identity
matrix passed as a host constant input).  Each core emits one f32 partial that
already includes the 1/(2*C*B) scaling; the host sums the 8 partials.
"""
import numpy as np
from contextlib import ExitStack

import concourse.bass as bass
import concourse.bacc as bacc
import concourse.tile as tile
from concourse import mybir
from concourse.alu_op_type import AluOpType
from concourse.bass import _add_dep_helper
from concourse.bass_utils import run_bass_kernel_spmd

B = 64            # graphs in the batch
PMAX = 1024       # edges per graph
N_CORES = 8
B_LOC = B // N_CORES            # 8 graphs per core
E_LOC = B_LOC * PMAX            # 8192 edges per core
CHUNKS = PMAX // 128            # 8 partition-chunks per graph
N_TILES = B_LOC * CHUNKS        # 64 tiles per core
PAIR_COUNT = PMAX * (PMAX - 1) // 2
SCALE = 1.0 / (2.0 * PAIR_COUNT * B)

F32 = mybir.dt.float32
BF16 = mybir.dt.bfloat16


def build_nc(reps: int = 1, loop_iters: int | None = None, variant: str = 'base') -> bacc.Bacc:
    """reps>1 unrolls the whole compute `reps` times; loop_iters=N wraps the
    main loop in a hardware For loop that runs it N times (same result; used
    to measure per-iteration HW time by wall-clock slope)."""
    nc = bacc.Bacc()
    x_ext = nc.declare_dram_parameter("x", [E_LOC], F32, isOutput=False)
    y_ext = nc.declare_dram_parameter("y", [E_LOC], F32, isOutput=False)
    out_ext = nc.declare_dram_parameter("out", [1, 1], F32, isOutput=True)

    with tile.TileContext(nc) as tc, ExitStack() as ctx:
        singles = ctx.enter_context(tc.tile_pool(name="singles", bufs=1))
        rows = ctx.enter_context(tc.tile_pool(name="rows", bufs=2))
        work = ctx.enter_context(tc.tile_pool(name="work", bufs=4))
        scratch = ctx.enter_context(tc.tile_pool(name="scratch", bufs=2))
        psum = ctx.enter_context(tc.tile_pool(name="psum", bufs=1, space="PSUM"))
        dram = ctx.enter_context(tc.tile_pool(name="dram", bufs=1, space="DRAM"))

        # ---- prologue: bf16 copies of x/y staged to DRAM scratch (source for
        # the per-graph broadcast-row DMAs)
        xbf_dram = dram.tile([B_LOC, PMAX], BF16)
        ybf_dram = dram.tile([B_LOC, PMAX], BF16)

        def stage_bf16(ext, bf_dram, tag):
            g8_f = singles.tile([B_LOC, PMAX], F32, tag=f"{tag}_g8f")
            nc.sync.dma_start(g8_f[:], ext[:].rearrange("(g n) -> g n", g=B_LOC))
            g8 = singles.tile([B_LOC, PMAX], BF16, tag=f"{tag}_g8")
            nc.vector.tensor_copy(g8[:], g8_f[:])
            nc.sync.dma_start(bf_dram[:], g8[:])
            return g8_f

        xg8_f_tile = yg8_f_tile = None
        if variant != "empty":
            xg8_f_tile = stage_bf16(x_ext, xbf_dram, "x")
            yg8_f_tile = stage_bf16(y_ext, ybf_dram, "y")

        # per-partition scalar columns, one [128, CHUNKS] f32 tile per graph:
        # xcol_g[p, r] = x[g*PMAX + 128*r + p]  (strided 4KB DMA from DRAM)
        xcols, ycols, xcols_bf = [], [], []
        for g in range(B_LOC if variant != "empty" else 0):
            xc = singles.tile([128, CHUNKS], F32, tag=f"xcol{g}")
            nc.sync.dma_start(
                xc[:], x_ext[g * PMAX:(g + 1) * PMAX].rearrange("(r p) -> p r", p=128))
            yc = singles.tile([128, CHUNKS], F32, tag=f"ycol{g}")
            nc.sync.dma_start(
                yc[:], y_ext[g * PMAX:(g + 1) * PMAX].rearrange("(r p) -> p r", p=128))
            xcols.append(xc)
            ycols.append(yc)
            if variant.startswith("mmB"):
                xcb = singles.tile([128, CHUNKS], BF16, tag=f"xcolbf{g}")
                nc.vector.tensor_copy(xcb[:], xc[:])
                xcols_bf.append(xcb)

        rcols = singles.tile([128, N_TILES], F32)
        if variant.startswith("mmB"):
            D_all = singles.tile([B_LOC, PMAX], F32)
            psB = psum.tile([1, PMAX], F32, tag="psB")
            ones8 = singles.tile([B_LOC, 1], F32)
            nc.vector.memset(ones8[:], 1.0)
            ones1 = singles.tile([1, 1], F32)
            nc.vector.memset(ones1[:], 1.0)
        if variant in ("norelu", "nott", "empty"):
            nc.vector.memset(rcols[:], 0.0)
        ones_bf = singles.tile([128, 1], BF16)
        nc.vector.memset(ones_bf[:], 1.0)
        # PSUM accumulator for sum_p of all gs tiles: [1, PMAX] f32
        if not variant.startswith("mmB"):
            psA = psum.tile([1, PMAX], F32)
        if variant in ("nott", "empty"):
            nc.vector.memset(psA[:], 0.0)

        # resident broadcast rows: all 8 graphs' X/Y rows live in SBUF
        Xrows, Yrows = [], []
        if variant not in ("dma_rows", "empty"):
            engs = [nc.sync, nc.scalar, nc.gpsimd]
            for g in range(B_LOC):
                Xr = singles.tile([128, PMAX], BF16, tag=f"Xrow{g}")
                engs[(2 * g) % len(engs)].dma_start(
                    Xr[:], xbf_dram[g:g + 1, :].partition_broadcast(128))
                Yr = singles.tile([128, PMAX], BF16, tag=f"Yrow{g}")
                engs[(2 * g + 1) % len(engs)].dma_start(
                    Yr[:], ybf_dram[g:g + 1, :].partition_broadcast(128))
                Xrows.append(Xr)
                Yrows.append(Yr)

        # ---- main loop: 8 graphs x 8 chunks (x reps)
        import contextlib
        loop_cm = (tc.For_i(0, loop_iters, 1) if loop_iters
                   else contextlib.nullcontext())
        with loop_cm:
            if variant == "empty":
                etile = work.tile([128, 1], F32, tag="etile")
                nc.vector.memset(etile[:], 0.0)
            for rep in range(reps):
                if variant == "empty":
                    break
                for g in range(B_LOC):
                    if variant == "dma_rows":
                        Xrow = rows.tile([128, PMAX], BF16, tag="Xrow")
                        nc.sync.dma_start(
                            Xrow[:], xbf_dram[g:g + 1, :].partition_broadcast(128))
                        Yrow = rows.tile([128, PMAX], BF16, tag="Yrow")
                        nc.sync.dma_start(
                            Yrow[:], ybf_dram[g:g + 1, :].partition_broadcast(128))
                    else:
                        Xrow, Yrow = Xrows[g], Yrows[g]
                    if variant.startswith("mmB"):
                        psD = psum.tile([1, PMAX], F32, tag="psD")
                    for r in range(CHUNKS):
                        t = g * CHUNKS + r
                        w = work.tile([128, PMAX], BF16, tag="w")
                        nc.vector.tensor_scalar(
                            w[:], Xrow[:], xcols[g][:, r:r + 1], None,
                            AluOpType.subtract)
                        h2 = work.tile([128, PMAX], BF16, tag="h2")
                        nc.vector.tensor_scalar(
                            h2[:], Yrow[:], ycols[g][:, r:r + 1], None,
                            AluOpType.is_gt)
                        if variant.startswith("mmB"):
                            # term B: sum_p x_p * H  (accumulate over ALL tiles)
                            # term A prep: D_g[f] = sum_p H[p, f]  (per graph)
                            for half in range(2):
                                sl = slice(half * 512, (half + 1) * 512)
                                nc.tensor.matmul(
                                    psB[:, sl], xcols_bf[g][:, r:r + 1], h2[:, sl],
                                    start=(t == 0), stop=(t == N_TILES - 1))
                                nc.tensor.matmul(
                                    psD[:, sl], ones_bf[:], h2[:, sl],
                                    start=(r == 0), stop=(r == CHUNKS - 1))
                        elif variant != "nott":
                            gs = scratch.tile([128, PMAX], BF16, tag="gs")
                            tt_eng = (nc.gpsimd if (variant == "ttg" and t % 2 == 0)
                                      else nc.vector)
                            tt_eng.tensor_tensor(gs[:], h2[:], w[:],
                                                 AluOpType.mult)
                            for half in range(2):
                                nc.tensor.matmul(
                                    psA[:, half * 512:(half + 1) * 512],
                                    ones_bf[:],
                                    gs[:, half * 512:(half + 1) * 512],
                                    start=(t == 0), stop=(t == N_TILES - 1))
                        if variant != "norelu":
                            rs = scratch.tile([128, PMAX], BF16, tag="rs")
                            if variant == "relu_v":
                                nc.vector.tensor_scalar(
                                    rs[:], w[:], 0.0, 0.0, AluOpType.max,
                                    AluOpType.add,
                                    accum_out=rcols[:, t:t + 1])
                            elif variant == "relu_g":
                                nc.gpsimd.tensor_scalar(
                                    rs[:], w[:], 0.0, 0.0, AluOpType.max,
                                    AluOpType.add,
                                    accum_out=rcols[:, t:t + 1])
                            elif variant == "relu_mix":
                                eng = nc.gpsimd if (t % 2 == 0) else nc.scalar
                                if eng is nc.scalar:
                                    nc.scalar.activation(
                                        rs[:], w[:],
                                        mybir.ActivationFunctionType.Relu,
                                        accum_out=rcols[:, t:t + 1])
                                else:
                                    nc.gpsimd.tensor_scalar(
                                        rs[:], w[:], 0.0, 0.0, AluOpType.max,
                                        AluOpType.add,
                                        accum_out=rcols[:, t:t + 1])
                            else:
                                nc.scalar.activation(
                                    rs[:], w[:],
                                    mybir.ActivationFunctionType.Relu,
                                    accum_out=rcols[:, t:t + 1])
                    if variant.startswith("mmB"):
                        nc.vector.tensor_copy(D_all[g:g + 1, :], psD[:])

        if variant.startswith("mmB"):
            # total = sum(rcols) + sum(psB) - sum_g dot(x_g, D_g), all * SCALE
            dsum = singles.tile([128, 1], F32)
            nc.vector.tensor_reduce(dsum[:], rcols[:], mybir.AxisListType.X,
                                    AluOpType.add)
            prod = singles.tile([B_LOC, PMAX], F32)
            nc.vector.tensor_tensor(prod[:], D_all[:], xg8_f_tile[:],
                                    AluOpType.mult)
            prodsum = singles.tile([B_LOC, 1], F32)
            nc.vector.tensor_reduce(prodsum[:], prod[:], mybir.AxisListType.X,
                                    AluOpType.add)
            prodneg = singles.tile([B_LOC, 1], F32)
            nc.vector.tensor_scalar(prodneg[:], prodsum[:], -1.0, None,
                                    AluOpType.mult)
            psBsum = singles.tile([1, 1], F32)
            nc.vector.tensor_reduce(psBsum[:], psB[:], mybir.AxisListType.X,
                                    AluOpType.add)
            ones = singles.tile([128, 1], F32)
            nc.vector.memset(ones[:], 1.0)
            ps = psum.tile([1, 1], F32)
            nc.tensor.matmul(ps[:], ones[:], dsum[:], start=True, stop=False)
            nc.tensor.matmul(ps[:], ones8[:], prodneg[:], start=False, stop=False)
            nc.tensor.matmul(ps[:], ones1[:], psBsum[:], start=False, stop=True)
            outsb = singles.tile([1, 1], F32)
            nc.scalar.activation(outsb[:], ps[:],
                                 mybir.ActivationFunctionType.Identity,
                                 scale=float(SCALE))
            nc.sync.dma_start(out_ext[:], outsb[:])
        else:
            # ---- epilogue: total = (sum(rcols) - sum(psA)) * SCALE
            dsum = singles.tile([128, 1], F32)
            nc.vector.tensor_reduce(dsum[:], rcols[:], mybir.AxisListType.X,
                                    AluOpType.add)
            ones = singles.tile([128, 1], F32)
            nc.vector.memset(ones[:], 1.0)
            ps = psum.tile([1, 1], F32)
            nc.tensor.matmul(ps[:], ones[:], dsum[:], start=True, stop=True)
            gtot = singles.tile([1, 1], F32)
            nc.vector.tensor_reduce(gtot[:], psA[:], mybir.AxisListType.X,
                                    AluOpType.add)
            rtot = singles.tile([1, 1], F32)
            nc.scalar.activation(rtot[:], ps[:],
                                 mybir.ActivationFunctionType.Identity)
            diff = singles.tile([1, 1], F32)
            nc.vector.tensor_tensor(diff[:], rtot[:], gtot[:], AluOpType.subtract)
            outsb = singles.tile([1, 1], F32)
            nc.scalar.activation(outsb[:], diff[:],
                                 mybir.ActivationFunctionType.Identity,
                                 scale=float(SCALE))
            nc.sync.dma_start(out_ext[:], outsb[:])

    nc.finalize()
    return nc


def build_nc2(reps: int = 1, loop_iters: int | None = None,
              accum_mod: int = 3, accum_keep: int = 2) -> bacc.Bacc:
    """Balanced-engine build: per tile
         V:  w = Xrow - x_col; h2 = [Yrow > y_col]; h2t = [Yrow < y_col]
         PE: psA2 += xcol_bf @ h2t ; psB += xcol_bf @ h2   (both Sum H*w terms)
         ACT: relu(w) with accum (accum_keep of accum_mod tiles) or plain relu
              + PE ones-matmul reduction for the rest
       total = sum(rcols) + sum(psR) + sum(psB) - sum(psA2), * SCALE.
    """
    nc = bacc.Bacc()
    x_ext = nc.declare_dram_parameter("x", [E_LOC], F32, isOutput=False)
    y_ext = nc.declare_dram_parameter("y", [E_LOC], F32, isOutput=False)
    ident_ext = nc.declare_dram_parameter("ident", [64, 64], F32, isOutput=False)
    out_ext = nc.declare_dram_parameter("out", [1, 1], F32, isOutput=True)

    with tile.TileContext(nc) as tc, ExitStack() as ctx:
        singles = ctx.enter_context(tc.tile_pool(name="singles", bufs=1))
        work = ctx.enter_context(tc.tile_pool(name="work", bufs=4))
        scratch = ctx.enter_context(tc.tile_pool(name="scratch", bufs=3))
        psum = ctx.enter_context(tc.tile_pool(name="psum", bufs=1, space="PSUM"))
        dram = ctx.enter_context(tc.tile_pool(name="dram", bufs=1, space="DRAM"))

        xbf_dram = dram.tile([B_LOC, PMAX], BF16)
        ybf_dram = dram.tile([B_LOC, PMAX], BF16)

        def stage_bf16(ext, bf_dram, tag):
            g8_f = singles.tile([B_LOC, PMAX], F32, tag=f"{tag}_g8f")
            nc.sync.dma_start(g8_f[:], ext[:].rearrange("(g n) -> g n", g=B_LOC))
            g8 = singles.tile([B_LOC, PMAX], BF16, tag=f"{tag}_g8")
            nc.vector.tensor_copy(g8[:], g8_f[:])
            nc.sync.dma_start(bf_dram[:], g8[:])
            return g8_f

        # per-partition scalar columns via PE transpose:
        # xin64 [64, 128] (straight) -> xcol_all [128, 64] with
        # xcol_all[p, t] = x[128 t + p]
        ident_sb = singles.tile([64, 64], F32)
        nc.sync.dma_start(ident_sb[:], ident_ext[:])
        xcol_all = singles.tile([128, 64], F32)
        ycol_all = singles.tile([128, 64], F32)
        xcol_all_bf = singles.tile([128, 64], BF16)
        for ext, dst, dst_bf, eng in ((x_ext, xcol_all, xcol_all_bf, nc.scalar),
                                      (y_ext, ycol_all, None, nc.gpsimd)):
            in64 = work.tile([64, 128], F32, tag="in64")
            eng.dma_start(in64[:], ext[:].rearrange("(c p) -> c p", p=128))
            psT = psum.tile([128, 64], F32, tag="psT")
            nc.tensor.matmul(psT[:], in64[:], ident_sb[:], is_transpose=True,
                             start=True, stop=True)
            nc.vector.tensor_copy(dst[:], psT[:])
            if dst_bf is not None:
                nc.vector.tensor_copy(dst_bf[:], psT[:])
        negxcol_all = singles.tile([128, 64], F32)
        nc.vector.tensor_scalar(negxcol_all[:], xcol_all[:], -1.0, None,
                                AluOpType.mult)

        xg8_f = stage_bf16(x_ext, xbf_dram, "x")
        stage_bf16(y_ext, ybf_dram, "y")

        # resident broadcast rows: one [128, E_LOC] tile per tensor, loaded
        # by 2 half DMAs each (128 contiguous-run descriptors per DMA)
        Xall = singles.tile([128, E_LOC], BF16)
        Yall = singles.tile([128, E_LOC], BF16)
        # graph-0 pieces first (small, unblock compute), then two big pieces
        pieces = [(0, 2 * PMAX), (2 * PMAX, E_LOC)]
        eng_rr = [nc.sync, nc.scalar, nc.gpsimd]
        k = 0
        for lo, hi in pieces:
            for src, dst in ((ybf_dram, Yall), (xbf_dram, Xall)):
                flat = src[:].rearrange("g n -> (g n)")
                eng_rr[k % 3].dma_start(
                    dst[:, lo:hi],
                    flat[lo:hi].unsqueeze(0).partition_broadcast(128))
                k += 1
        Xrows = [Xall[:, g * PMAX:(g + 1) * PMAX] for g in range(B_LOC)]
        Yrows = [Yall[:, g * PMAX:(g + 1) * PMAX] for g in range(B_LOC)]

        rcols = singles.tile([128, N_TILES], F32)
        nc.vector.memset(rcols[:], 0.0)
        ones_bf = singles.tile([128, 1], BF16)
        nc.vector.memset(ones_bf[:], 1.0)
        psB = psum.tile([1, PMAX], F32, tag="psB")
        psR = psum.tile([1, PMAX], F32, tag="psR")

        import contextlib
        loop_cm = (tc.For_i(0, loop_iters, 1) if loop_iters
                   else contextlib.nullcontext())
        n_acc = 0
        n_mm = 0
        mm_ts = [t for t in range(N_TILES) if t % accum_mod < accum_keep]
        last_mm_t = mm_ts[-1] if mm_ts else None
        with loop_cm:
            for rep in range(reps):
                first = (rep == 0)
                last = (rep == reps - 1)
                for g in range(B_LOC):
                    Xrow, Yrow = Xrows[g], Yrows[g]
                    for r in range(CHUNKS):
                        t = g * CHUNKS + r
                        c = 8 * g + r
                        h2 = work.tile([128, PMAX], BF16, tag="h2")
                        nc.vector.tensor_scalar(
                            h2[:], Yrow[:], ycol_all[:, c:c + 1],
                            None, AluOpType.is_gt)
                        for half in range(2):
                            sl = slice(half * 512, (half + 1) * 512)
                            nc.tensor.matmul(
                                psB[:, sl], xcol_all_bf[:, c:c + 1], h2[:, sl],
                                start=(first and t == 0),
                                stop=(last and t == N_TILES - 1))
                        rl = scratch.tile([128, PMAX], BF16, tag="rl")
                        if t % accum_mod < accum_keep:
                            # rl = relu(Xrow - x_col) in one fused DVE op
                            nc.vector.tensor_scalar(
                                rl[:], Xrow[:], xcol_all[:, c:c + 1], 0.0,
                                AluOpType.subtract, AluOpType.max)
                            for half in range(2):
                                sl = slice(half * 512, (half + 1) * 512)
                                nc.tensor.matmul(
                                    psR[:, sl], ones_bf[:], rl[:, sl],
                                    start=(first and t == mm_ts[0]),
                                    stop=(last and t == last_mm_t))
                            n_mm += 1
                        else:
                            # relu(Xrow + (-x_col)) + accum directly on ScalarE
                            nc.scalar.activation(
                                rl[:], Xrow[:],
                                mybir.ActivationFunctionType.Relu,
                                bias=negxcol_all[:, c:c + 1], scale=1.0,
                                accum_out=rcols[:, t:t + 1])


        # epilogue: total = sum(rcols) + sum(psR) + 2*sum(psB) - 1023*sum(x)
        dsum = singles.tile([128, 1], F32)
        nc.vector.tensor_reduce(dsum[:], rcols[:], mybir.AxisListType.X,
                                AluOpType.add)
        ones128e = singles.tile([128, 1], F32)
        nc.vector.memset(ones128e[:], 1.0)
        psum_r = singles.tile([1, 1], F32)
        nc.vector.tensor_reduce(psum_r[:], psR[:], mybir.AxisListType.X,
                                AluOpType.add)
        psum_b = singles.tile([1, 1], F32)
        dummy_b = singles.tile([1, PMAX], F32)
        nc.scalar.activation(dummy_b[:], psB[:],
                             mybir.ActivationFunctionType.Identity,
                             accum_out=psum_b[:])
        xsum8 = singles.tile([B_LOC, 1], F32)
        nc.vector.tensor_reduce(xsum8[:], xg8_f[:], mybir.AxisListType.X,
                                AluOpType.add)
        xsum8n = singles.tile([B_LOC, 1], F32)
        nc.vector.tensor_scalar(xsum8n[:], xsum8[:], -float(PMAX - 1), None,
                                AluOpType.mult)
        c1 = singles.tile([1, 1], F32)
        nc.vector.tensor_scalar(c1[:], psum_b[:], 2.0, None, AluOpType.mult)
        c2 = singles.tile([1, 1], F32)
        nc.vector.tensor_tensor(c2[:], c1[:], psum_r[:], AluOpType.add)
        ones8e = singles.tile([B_LOC, 1], F32)
        nc.vector.memset(ones8e[:], 1.0)
        ones1 = singles.tile([1, 1], F32)
        nc.vector.memset(ones1[:], 1.0)
        ps = psum.tile([1, 1], F32, tag="psfin")
        nc.tensor.matmul(ps[:], ones128e[:], dsum[:], start=True, stop=False)
        nc.tensor.matmul(ps[:], ones8e[:], xsum8n[:], start=False, stop=False)
        nc.tensor.matmul(ps[:], ones1[:], c2[:], start=False, stop=True)
        outsb = singles.tile([1, 1], F32)
        nc.scalar.activation(outsb[:], ps[:],
                             mybir.ActivationFunctionType.Identity,
                             scale=float(SCALE))
        nc.sync.dma_start(out_ext[:], outsb[:])

    nc.finalize()
    return nc


def build_nc3(reps: int = 1, loop_iters: int | None = None,
              n_act: int = 12, n_gps: int = 9, act0: int = 26,
              use_bf16: bool = False) -> bacc.Bacc:
    """Rank-identity build: the whole loss collapses to
         total = sum_g sum_i x_i * (rank_x(i) - rank_y(i)) / (C*B),
       and ranks are row-sums of comparison tiles, obtained for free via
       accum_out.  Main loop = 128 elementwise ops (64 tiles x {x,y}), NO
       matmuls / PSUM at all:
         DVE : out = is_gt(Row, col)  (tensor_scalar fp16, 4x mode, accum)
         ACT : out = Sign(Row - col)  (activation, accum)  [affine of rank]
         GPS : out = is_gt(Row, col)  (tensor_scalar, accum)
       Rows are fp16; compare scalars are fp16-rounded (so the self-compare
       is exactly 0).  Epilogue: D = (raccY - raccX) * wAx, reduce, scale.
       Tiles [act0, act0+n_act) go to ScalarE (Sign), the last n_gps to
       GpSimd, the rest to DVE."""
    nc = bacc.Bacc()
    x_ext = nc.declare_dram_parameter("x", [E_LOC], F32, isOutput=False)
    y_ext = nc.declare_dram_parameter("y", [E_LOC], F32, isOutput=False)
    ident_ext = nc.declare_dram_parameter("ident", [64, 64], F32, isOutput=False)
    out_ext = nc.declare_dram_parameter("out", [1, 1], F32, isOutput=True)

    FP16 = BF16 if use_bf16 else mybir.dt.float16
    act_ts = set(range(act0, act0 + n_act))
    gps_ts = set(range(N_TILES - n_gps, N_TILES))

    with tile.TileContext(nc) as tc, ExitStack() as ctx:
        singles = ctx.enter_context(tc.tile_pool(name="singles", bufs=1))
        work = ctx.enter_context(tc.tile_pool(name="work", bufs=8))
        ascr = ctx.enter_context(tc.tile_pool(name="ascr", bufs=4))
        gscr = ctx.enter_context(tc.tile_pool(name="gscr", bufs=4))
        psum = ctx.enter_context(tc.tile_pool(name="psum", bufs=1, space="PSUM"))
        dram = ctx.enter_context(tc.tile_pool(name="dram", bufs=1, space="DRAM"))

        # ---- per-partition scalar columns via PE transpose (as build_nc2)
        ident_sb = singles.tile([64, 64], F32)
        nc.sync.dma_start(ident_sb[:], ident_ext[:])
        xcol_all = singles.tile([128, 64], F32)   # exact f32 (epilogue weight)
        xcol_r = singles.tile([128, 64], F32)     # fp16-rounded compare scalar
        ycol_r = singles.tile([128, 64], F32)
        negx_r = singles.tile([128, 64], F32)     # -rounded (ACT Sign bias)
        negy_r = singles.tile([128, 64], F32)
        col16 = singles.tile([128, 64], FP16)
        for ext, dst_f32, dst_r, dst_neg, eng in (
                (x_ext, xcol_all, xcol_r, negx_r, nc.scalar),
                (y_ext, None, ycol_r, negy_r, nc.gpsimd)):
            in64 = work.tile([64, 128], F32, tag="in64")
            eng.dma_start(in64[:], ext[:].rearrange("(c p) -> c p", p=128))
            psT = psum.tile([128, 64], F32, tag="psT")
            nc.tensor.matmul(psT[:], in64[:], ident_sb[:], is_transpose=True,
                             start=True, stop=True)
            if dst_f32 is not None:
                nc.vector.tensor_copy(dst_f32[:], psT[:])
            nc.vector.tensor_copy(col16[:], psT[:])       # round to fp16
            nc.vector.tensor_copy(dst_r[:], col16[:])     # back to exact f32
            nc.vector.tensor_scalar(dst_neg[:], dst_r[:], -1.0, None,
                                    AluOpType.mult)

        # ---- fp16 broadcast rows (stage fp16 to DRAM, then bcast-DMA)
        x16_dram = dram.tile([B_LOC, PMAX], FP16)
        y16_dram = dram.tile([B_LOC, PMAX], FP16)
        for ext, d16, tag in ((x_ext, x16_dram, "x"), (y_ext, y16_dram, "y")):
            g8_f = work.tile([B_LOC, PMAX], F32, tag=f"{tag}_g8f")
            nc.sync.dma_start(g8_f[:], ext[:].rearrange("(g n) -> g n", g=B_LOC))
            g8 = work.tile([B_LOC, PMAX], FP16, tag=f"{tag}_g8")
            nc.vector.tensor_copy(g8[:], g8_f[:])
            nc.sync.dma_start(d16[:], g8[:])
        Xall = singles.tile([128, E_LOC], FP16)
        Yall = singles.tile([128, E_LOC], FP16)
        pieces = [(0, 2 * PMAX), (2 * PMAX, E_LOC)]
        eng_rr = [nc.sync, nc.scalar, nc.gpsimd]
        k = 0
        for lo, hi in pieces:
            for src, dst in ((y16_dram, Yall), (x16_dram, Xall)):
                flat = src[:].rearrange("g n -> (g n)")
                eng_rr[k % 3].dma_start(
                    dst[:, lo:hi],
                    flat[lo:hi].unsqueeze(0).partition_broadcast(128))
                k += 1
        Xrows = [Xall[:, g * PMAX:(g + 1) * PMAX] for g in range(B_LOC)]
        Yrows = [Yall[:, g * PMAX:(g + 1) * PMAX] for g in range(B_LOC)]

        # epilogue weight: wAx[p,t] = x_exact * (+1 | +0.5 for ACT tiles)
        wAx = singles.tile([128, 64], F32)
        nc.vector.tensor_copy(wAx[:], xcol_all[:])
        if n_act:
            nc.vector.tensor_scalar(wAx[:, act0:act0 + n_act],
                                    xcol_all[:, act0:act0 + n_act],
                                    0.5, None, AluOpType.mult)

        raccX = singles.tile([128, N_TILES], F32)
        raccY = singles.tile([128, N_TILES], F32)

        import contextlib
        loop_cm = (tc.For_i(0, loop_iters, 1) if loop_iters
                   else contextlib.nullcontext())
        with loop_cm:
            for rep in range(reps):
                for t in range(N_TILES):
                    g, a = divmod(t, CHUNKS)
                    Xrow, Yrow = Xrows[g], Yrows[g]
                    for (Row, colr, negr, racc) in (
                            (Xrow, xcol_r, negx_r, raccX),
                            (Yrow, ycol_r, negy_r, raccY)):
                        if t in act_ts:
                            scr = ascr.tile([128, PMAX], FP16, tag="ascr")
                            nc.scalar.activation(
                                scr[:], Row[:],
                                mybir.ActivationFunctionType.Sign,
                                bias=negr[:, t:t + 1], scale=1.0,
                                accum_out=racc[:, t:t + 1])
                        elif t in gps_ts:
                            scr = gscr.tile([128, PMAX], FP16, tag="gscr")
                            nc.gpsimd.tensor_scalar(
                                scr[:], Row[:], colr[:, t:t + 1], 0.0,
                                AluOpType.is_gt, AluOpType.add,
                                accum_out=racc[:, t:t + 1])
                        else:
                            scr = work.tile([128, PMAX], FP16, tag="wscr")
                            nc.vector.tensor_scalar(
                                scr[:], Row[:], colr[:, t:t + 1], 0.0,
                                AluOpType.is_gt, AluOpType.add,
                                accum_out=racc[:, t:t + 1])

        # ---- epilogue: total = sum_p sum_t wAx * (raccY - raccX), * SCALE3
        D = singles.tile([128, N_TILES], F32)
        nc.vector.tensor_tensor(D[:], raccY[:], raccX[:], AluOpType.subtract)
        P = singles.tile([128, N_TILES], F32)
        nc.vector.tensor_tensor(P[:], D[:], wAx[:], AluOpType.mult)
        dsum = singles.tile([128, 1], F32)
        nc.vector.tensor_reduce(dsum[:], P[:], mybir.AxisListType.X,
                                AluOpType.add)
        ones128e = singles.tile([128, 1], F32)
        nc.vector.memset(ones128e[:], 1.0)
        ps = psum.tile([1, 1], F32, tag="psfin")
        nc.tensor.matmul(ps[:], ones128e[:], dsum[:], start=True, stop=True)
        outsb = singles.tile([1, 1], F32)
        nc.scalar.activation(outsb[:], ps[:],
                             mybir.ActivationFunctionType.Identity,
                             scale=float(1.0 / (PAIR_COUNT * B)))
        nc.sync.dma_start(out_ext[:], outsb[:])

    nc.finalize()
    return nc


def build_nc5(reps: int = 1, loop_iters: int | None = None,
              k_relu: int = 1) -> bacc.Bacc:
    """Mixed-form build: graphs 0..k_relu-1 use the relu+H full-matrix
    identity with the relu side on ScalarE (Relu+accum, proven fast);
    remaining graphs use the rank identity all on DVE (fp16 is_gt+accum).
      relu-graph g:  S_g = 0.5*sum(RL) - 511.5*sum(x_g) + sum(x*gt)
      rank-graph g:  S_g = sum(x*(gtY - gtX))
    Epilogue: total = sum(A.raccX + B.raccY + C) with per-column weights."""
    nc = bacc.Bacc()
    x_ext = nc.declare_dram_parameter("x", [E_LOC], F32, isOutput=False)
    y_ext = nc.declare_dram_parameter("y", [E_LOC], F32, isOutput=False)
    ident_ext = nc.declare_dram_parameter("ident", [64, 64], F32, isOutput=False)
    out_ext = nc.declare_dram_parameter("out", [1, 1], F32, isOutput=True)

    FP16 = mybir.dt.float16
    relu_ts = set(range(0, k_relu * CHUNKS))   # tiles of the relu-form graphs

    with tile.TileContext(nc) as tc, ExitStack() as ctx:
        singles = ctx.enter_context(tc.tile_pool(name="singles", bufs=1))
        work = ctx.enter_context(tc.tile_pool(name="work", bufs=8))
        ascr = ctx.enter_context(tc.tile_pool(name="ascr", bufs=4))
        psum = ctx.enter_context(tc.tile_pool(name="psum", bufs=1, space="PSUM"))
        dram = ctx.enter_context(tc.tile_pool(name="dram", bufs=1, space="DRAM"))

        ident_sb = singles.tile([64, 64], F32)
        nc.sync.dma_start(ident_sb[:], ident_ext[:])
        xcol_all = singles.tile([128, 64], F32)
        xcol_r = singles.tile([128, 64], F32)
        ycol_r = singles.tile([128, 64], F32)
        negx_r = singles.tile([128, 64], F32)
        negy_r = singles.tile([128, 64], F32)
        col16 = singles.tile([128, 64], FP16)
        for ext, dst_f32, dst_r, dst_neg, eng in (
                (x_ext, xcol_all, xcol_r, negx_r, nc.scalar),
                (y_ext, None, ycol_r, negy_r, nc.gpsimd)):
            in64 = work.tile([64, 128], F32, tag="in64")
            eng.dma_start(in64[:], ext[:].rearrange("(c p) -> c p", p=128))
            psT = psum.tile([128, 64], F32, tag="psT")
            nc.tensor.matmul(psT[:], in64[:], ident_sb[:], is_transpose=True,
                             start=True, stop=True)
            if dst_f32 is not None:
                nc.vector.tensor_copy(dst_f32[:], psT[:])
            nc.vector.tensor_copy(col16[:], psT[:])
            nc.vector.tensor_copy(dst_r[:], col16[:])
            nc.vector.tensor_scalar(dst_neg[:], dst_r[:], -1.0, None,
                                    AluOpType.mult)

        x16_dram = dram.tile([B_LOC, PMAX], FP16)
        y16_dram = dram.tile([B_LOC, PMAX], FP16)
        for ext, d16, tag in ((x_ext, x16_dram, "x"), (y_ext, y16_dram, "y")):
            g8_f = work.tile([B_LOC, PMAX], F32, tag=f"{tag}_g8f")
            nc.sync.dma_start(g8_f[:], ext[:].rearrange("(g n) -> g n", g=B_LOC))
            g8 = work.tile([B_LOC, PMAX], FP16, tag=f"{tag}_g8")
            nc.vector.tensor_copy(g8[:], g8_f[:])
            nc.sync.dma_start(d16[:], g8[:])
        Xall = singles.tile([128, E_LOC], FP16)
        Yall = singles.tile([128, E_LOC], FP16)
        pieces = [(0, 2 * PMAX), (2 * PMAX, E_LOC)]
        eng_rr = [nc.sync, nc.scalar, nc.gpsimd]
        k = 0
        for lo, hi in pieces:
            for src, dst in ((y16_dram, Yall), (x16_dram, Xall)):
                flat = src[:].rearrange("g n -> (g n)")
                eng_rr[k % 3].dma_start(
                    dst[:, lo:hi],
                    flat[lo:hi].unsqueeze(0).partition_broadcast(128))
                k += 1
        Xrows = [Xall[:, g * PMAX:(g + 1) * PMAX] for g in range(B_LOC)]
        Yrows = [Yall[:, g * PMAX:(g + 1) * PMAX] for g in range(B_LOC)]

        # per-column epilogue weights
        nr = k_relu * CHUNKS
        A = singles.tile([128, N_TILES], F32)     # multiplies raccX
        C = singles.tile([128, N_TILES], F32)     # additive constant
        nc.vector.memset(C[:], 0.0)
        nc.vector.tensor_scalar(A[:], xcol_all[:], -1.0, None, AluOpType.mult)
        if nr:
            nc.vector.memset(A[:, 0:nr], 0.5)
            nc.vector.tensor_scalar(C[:, 0:nr], xcol_all[:, 0:nr],
                                    -511.5, None, AluOpType.mult)

        raccX = singles.tile([128, N_TILES], F32)
        raccY = singles.tile([128, N_TILES], F32)

        import contextlib
        loop_cm = (tc.For_i(0, loop_iters, 1) if loop_iters
                   else contextlib.nullcontext())
        with loop_cm:
            for rep in range(reps):
                for t in range(N_TILES):
                    g, a = divmod(t, CHUNKS)
                    # x-side op
                    if t in relu_ts:
                        scr = ascr.tile([128, PMAX], FP16, tag="ascr")
                        nc.scalar.activation(
                            scr[:], Xrows[g][:],
                            mybir.ActivationFunctionType.Relu,
                            bias=negx_r[:, t:t + 1], scale=1.0,
                            accum_out=raccX[:, t:t + 1])
                    else:
                        scr = work.tile([128, PMAX], FP16, tag="wscr")
                        nc.vector.tensor_scalar(
                            scr[:], Xrows[g][:], xcol_r[:, t:t + 1], 0.0,
                            AluOpType.is_gt, AluOpType.add,
                            accum_out=raccX[:, t:t + 1])
                    # y-side op: always DVE is_gt
                    scr = work.tile([128, PMAX], FP16, tag="wscr")
                    nc.vector.tensor_scalar(
                        scr[:], Yrows[g][:], ycol_r[:, t:t + 1], 0.0,
                        AluOpType.is_gt, AluOpType.add,
                        accum_out=raccY[:, t:t + 1])

        # epilogue: total = sum(A*raccX + x*raccY + C) * SCALE3
        E1 = singles.tile([128, N_TILES], F32)
        nc.vector.tensor_tensor(E1[:], A[:], raccX[:], AluOpType.mult)
        E2 = singles.tile([128, N_TILES], F32)
        nc.vector.tensor_tensor(E2[:], xcol_all[:], raccY[:], AluOpType.mult)
        E3 = singles.tile([128, N_TILES], F32)
        nc.vector.tensor_tensor(E3[:], E1[:], E2[:], AluOpType.add)
        E4 = singles.tile([128, N_TILES], F32)
        nc.vector.tensor_tensor(E4[:], E3[:], C[:], AluOpType.add)
        dsum = singles.tile([128, 1], F32)
        nc.vector.tensor_reduce(dsum[:], E4[:], mybir.AxisListType.X,
                                AluOpType.add)
        ones128e = singles.tile([128, 1], F32)
        nc.vector.memset(ones128e[:], 1.0)
        ps = psum.tile([1, 1], F32, tag="psfin")
        nc.tensor.matmul(ps[:], ones128e[:], dsum[:], start=True, stop=True)
        outsb = singles.tile([1, 1], F32)
        nc.scalar.activation(outsb[:], ps[:],
                             mybir.ActivationFunctionType.Identity,
                             scale=float(1.0 / (PAIR_COUNT * B)))
        nc.sync.dma_start(out_ext[:], outsb[:])

    nc.finalize()
    return nc


def build_nc4(reps: int = 1, loop_iters: int | None = None,
              n_act: int = 0, use_bf16: bool = False) -> bacc.Bacc:
    """Rank-identity, PE-consumption build: DVE emits is_gt tiles with NO
    accum; TensorE consumes each with a +/-xcol 1-row matmul accumulated
    into one [1, PMAX] PSUM chain (psS += xcol^T @ h_y - xcol^T @ h_x).
    ACT tiles (first n_act) still use Sign+accum (they self-reduce).
    total = sum(psS) + act-part, * 1/(C*B)."""
    nc = bacc.Bacc()
    x_ext = nc.declare_dram_parameter("x", [E_LOC], F32, isOutput=False)
    y_ext = nc.declare_dram_parameter("y", [E_LOC], F32, isOutput=False)
    ident_ext = nc.declare_dram_parameter("ident", [64, 64], F32, isOutput=False)
    out_ext = nc.declare_dram_parameter("out", [1, 1], F32, isOutput=True)

    DT = BF16 if use_bf16 else mybir.dt.float16
    act_ts = set(range(0, n_act))

    with tile.TileContext(nc) as tc, ExitStack() as ctx:
        singles = ctx.enter_context(tc.tile_pool(name="singles", bufs=1))
        work = ctx.enter_context(tc.tile_pool(name="work", bufs=8))
        ascr = ctx.enter_context(tc.tile_pool(name="ascr", bufs=4))
        psum = ctx.enter_context(tc.tile_pool(name="psum", bufs=1, space="PSUM"))
        dram = ctx.enter_context(tc.tile_pool(name="dram", bufs=1, space="DRAM"))

        ident_sb = singles.tile([64, 64], F32)
        nc.sync.dma_start(ident_sb[:], ident_ext[:])
        xcol_all = singles.tile([128, 64], F32)
        xcol_r = singles.tile([128, 64], F32)
        ycol_r = singles.tile([128, 64], F32)
        negx_r = singles.tile([128, 64], F32)
        negy_r = singles.tile([128, 64], F32)
        col16 = singles.tile([128, 64], DT)
        for ext, dst_f32, dst_r, dst_neg, eng in (
                (x_ext, xcol_all, xcol_r, negx_r, nc.scalar),
                (y_ext, None, ycol_r, negy_r, nc.gpsimd)):
            in64 = work.tile([64, 128], F32, tag="in64")
            eng.dma_start(in64[:], ext[:].rearrange("(c p) -> c p", p=128))
            psT = psum.tile([128, 64], F32, tag="psT")
            nc.tensor.matmul(psT[:], in64[:], ident_sb[:], is_transpose=True,
                             start=True, stop=True)
            if dst_f32 is not None:
                nc.vector.tensor_copy(dst_f32[:], psT[:])
            nc.vector.tensor_copy(col16[:], psT[:])
            nc.vector.tensor_copy(dst_r[:], col16[:])
            nc.vector.tensor_scalar(dst_neg[:], dst_r[:], -1.0, None,
                                    AluOpType.mult)
        # matmul weight columns: +/- x in the row dtype (rounded)
        xw = singles.tile([128, 64], DT)
        nc.vector.tensor_copy(xw[:], xcol_all[:])
        xwn = singles.tile([128, 64], DT)
        nc.vector.tensor_scalar(xwn[:], xcol_all[:], -1.0, None,
                                AluOpType.mult)

        x16_dram = dram.tile([B_LOC, PMAX], DT)
        y16_dram = dram.tile([B_LOC, PMAX], DT)
        for ext, d16, tag in ((x_ext, x16_dram, "x"), (y_ext, y16_dram, "y")):
            g8_f = work.tile([B_LOC, PMAX], F32, tag=f"{tag}_g8f")
            nc.sync.dma_start(g8_f[:], ext[:].rearrange("(g n) -> g n", g=B_LOC))
            g8 = work.tile([B_LOC, PMAX], DT, tag=f"{tag}_g8")
            nc.vector.tensor_copy(g8[:], g8_f[:])
            nc.sync.dma_start(d16[:], g8[:])
        Xall = singles.tile([128, E_LOC], DT)
        Yall = singles.tile([128, E_LOC], DT)
        pieces = [(0, 2 * PMAX), (2 * PMAX, E_LOC)]
        eng_rr = [nc.sync, nc.scalar, nc.gpsimd]
        k = 0
        for lo, hi in pieces:
            for src, dst in ((y16_dram, Yall), (x16_dram, Xall)):
                flat = src[:].rearrange("g n -> (g n)")
                eng_rr[k % 3].dma_start(
                    dst[:, lo:hi],
                    flat[lo:hi].unsqueeze(0).partition_broadcast(128))
                k += 1
        Xrows = [Xall[:, g * PMAX:(g + 1) * PMAX] for g in range(B_LOC)]
        Yrows = [Yall[:, g * PMAX:(g + 1) * PMAX] for g in range(B_LOC)]

        # ACT part epilogue weight (0.5*x on ACT columns, else 0)
        raccX = singles.tile([128, N_TILES], F32)
        raccY = singles.tile([128, N_TILES], F32)
        if n_act:
            nc.vector.memset(raccX[:], 0.0)
            nc.vector.memset(raccY[:], 0.0)
        psS = psum.tile([1, PMAX], F32, tag="psS")

        n_pe = (N_TILES - n_act) * 2  # matmul-consumed op count

        import contextlib
        loop_cm = (tc.For_i(0, loop_iters, 1) if loop_iters
                   else contextlib.nullcontext())
        with loop_cm:
            for rep in range(reps):
                k_mm = 0
                for t in range(N_TILES):
                    g, a = divmod(t, CHUNKS)
                    for (Row, colr, negr, racc, w) in (
                            (Xrows[g], xcol_r, negx_r, raccX, xwn),
                            (Yrows[g], ycol_r, negy_r, raccY, xw)):
                        if t in act_ts:
                            scr = ascr.tile([128, PMAX], DT, tag="ascr")
                            nc.scalar.activation(
                                scr[:], Row[:],
                                mybir.ActivationFunctionType.Sign,
                                bias=negr[:, t:t + 1], scale=1.0,
                                accum_out=racc[:, t:t + 1])
                        else:
                            scr = work.tile([128, PMAX], DT, tag="wscr")
                            nc.vector.tensor_scalar(
                                scr[:], Row[:], colr[:, t:t + 1], None,
                                AluOpType.is_gt)
                            for half in range(2):
                                sl = slice(half * 512, (half + 1) * 512)
                                nc.tensor.matmul(
                                    psS[:, sl], w[:, t:t + 1], scr[:, sl],
                                    start=(k_mm == 0),
                                    stop=(k_mm == n_pe - 1))
                            k_mm += 1

        # epilogue: total = sum(psS) + 0.5 * sum(x * (ssY - ssX) on ACT cols)
        tot = singles.tile([1, 1], F32)
        dummy = singles.tile([1, PMAX], F32)
        nc.scalar.activation(dummy[:], psS[:],
                             mybir.ActivationFunctionType.Identity,
                             accum_out=tot[:])
        if n_act:
            D = singles.tile([128, N_TILES], F32)
            nc.vector.tensor_tensor(D[:], raccY[:], raccX[:],
                                    AluOpType.subtract)
            P = singles.tile([128, N_TILES], F32)
            nc.vector.tensor_tensor(P[:], D[:], xcol_all[:], AluOpType.mult)
            dsum = singles.tile([128, 1], F32)
            nc.vector.tensor_reduce(dsum[:], P[:], mybir.AxisListType.X,
                                    AluOpType.add)
            ones128e = singles.tile([128, 1], F32)
            nc.vector.memset(ones128e[:], 1.0)
            psA = psum.tile([1, 1], F32, tag="psA")
            nc.tensor.matmul(psA[:], ones128e[:], dsum[:], start=True,
                             stop=True)
            half_act = singles.tile([1, 1], F32)
            nc.scalar.activation(half_act[:], psA[:],
                                 mybir.ActivationFunctionType.Identity,
                                 scale=0.5)
            tot2 = singles.tile([1, 1], F32)
            nc.vector.tensor_tensor(tot2[:], tot[:], half_act[:],
                                    AluOpType.add)
            tot = tot2
        outsb = singles.tile([1, 1], F32)
        nc.scalar.activation(outsb[:], tot[:],
                             mybir.ActivationFunctionType.Identity,
                             scale=float(1.0 / (PAIR_COUNT * B)))
        nc.sync.dma_start(out_ext[:], outsb[:])

    nc.finalize()
    return nc


class _Runner:
    """Persistent compiled executor for the SPMD bass program: traces and
    compiles the jit once, then each call is just a dispatch. Mirrors
    concourse.bass2jax.run_bass_via_pjrt's multi-core branch."""

    def __init__(self, nc):
        import jax
        from jax.experimental.shard_map import shard_map
        from jax.sharding import Mesh, PartitionSpec
        from concourse import bass2jax

        bass2jax.install_neuronx_cc_hook()
        self.nc = nc
        in_names, out_names, out_avals, zero_outs = [], [], [], []
        partition_name = (nc.partition_id_tensor.name
                          if nc.partition_id_tensor else None)
        for alloc in nc.m.functions[0].allocations:
            if not isinstance(alloc, mybir.MemoryLocationSet):
                continue
            name = alloc.memorylocations[0].name
            if alloc.kind == "ExternalInput":
                if name != partition_name:
                    in_names.append(name)
            elif alloc.kind == "ExternalOutput":
                shape = tuple(alloc.tensor_shape)
                dtype = mybir.dt.np(alloc.dtype)
                out_names.append(name)
                out_avals.append(jax.core.ShapedArray(shape, dtype))
                zero_outs.append(np.zeros(shape, dtype))
        n_params = len(in_names)
        n_outs = len(out_avals)
        all_in_names = list(in_names) + list(out_names)
        if partition_name is not None:
            all_in_names.append(partition_name)
        self.in_names = in_names
        self.out_names = out_names
        self.zero_outs = zero_outs
        donate = tuple(range(n_params, n_params + n_outs))

        def _body(*args):
            operands = list(args)
            if partition_name is not None:
                operands.append(bass2jax.partition_id_tensor())
            outs = bass2jax._bass_exec_p.bind(
                *operands,
                out_avals=tuple(out_avals),
                in_names=tuple(all_in_names),
                out_names=tuple(out_names),
                lowering_input_output_aliases=(),
                sim_require_finite=True,
                sim_require_nnan=True,
                nc=nc,
            )
            return tuple(outs)

        devices = jax.devices()[:N_CORES]
        assert len(devices) == N_CORES
        mesh = Mesh(np.asarray(devices), ("core",))
        in_specs = (PartitionSpec("core"),) * (n_params + n_outs)
        out_specs = (PartitionSpec("core"),) * n_outs
        self._jit = jax.jit(
            shard_map(_body, mesh=mesh, in_specs=in_specs, out_specs=out_specs,
                      check_rep=False),
            donate_argnums=donate, keep_unused=True)

    def __call__(self, in_maps):
        import jax
        if "ident" in self.in_names and "ident" not in in_maps[0]:
            eye = np.eye(64, dtype=np.float32)
            in_maps = [{**m, "ident": eye} for m in in_maps]
        concat_in = [
            np.concatenate([np.asarray(in_maps[c][k]) for c in range(N_CORES)],
                           axis=0)
            for k in self.in_names
        ]
        zeros = [np.concatenate([z] * N_CORES, axis=0) for z in self.zero_outs]
        outs = self._jit(*concat_in, *zeros)
        outs = [np.asarray(o) for o in jax.block_until_ready(outs)]
        res = []
        for c in range(N_CORES):
            m = {}
            for i, name in enumerate(self.out_names):
                n0 = self.zero_outs[i].shape[0]
                m[name] = outs[i][c * n0:(c + 1) * n0]
            res.append(m)
        return res


_RUNNERS: dict = {}


def get_runner(reps: int = 1, loop_iters: int | None = None,
               variant: str = "base") -> _Runner:
    key = (reps, loop_iters, variant)
    if key not in _RUNNERS:
        if variant.startswith("r3"):
            parts = variant.split("_")
            na = int(parts[1]) if len(parts) > 1 else 12
            ng = int(parts[2]) if len(parts) > 2 else 9
            a0 = int(parts[3]) if len(parts) > 3 else 26
            _RUNNERS[key] = _Runner(build_nc3(
                reps, loop_iters, na, ng, a0,
                use_bf16=variant.startswith("r3b")))
        elif variant.startswith("v2"):
            parts = variant.split("_")
            am = int(parts[1]) if len(parts) > 2 else 3
            ak = int(parts[2]) if len(parts) > 2 else 2
            _RUNNERS[key] = _Runner(build_nc2(reps, loop_iters, am, ak))
        else:
            _RUNNERS[key] = _Runner(build_nc(reps, loop_iters, variant))
    return _RUNNERS[key]


def kernel(outputs: np.ndarray, y: np.ndarray, edges_batch: np.ndarray) -> np.ndarray:
    outputs = np.ascontiguousarray(np.asarray(outputs, dtype=np.float32))
    y = np.ascontiguousarray(np.asarray(y, dtype=np.float32))
    eb = np.asarray(edges_batch)
    assert outputs.shape == (B * PMAX,) and y.shape == (B * PMAX,)
    # this kernel is specialized to the PyG-style equal-sized-graph batch the
    # problem generates: edges_batch == repeat(arange(B), PMAX)
    expected_eb = np.repeat(np.arange(B, dtype=eb.dtype), PMAX)
    assert np.array_equal(eb, expected_eb), "kernel requires equal-sized graphs"

    in_maps = [
        {"x": outputs[i * E_LOC:(i + 1) * E_LOC], "y": y[i * E_LOC:(i + 1) * E_LOC]}
        for i in range(N_CORES)
    ]
    res = get_runner(1, variant="v2_2_1")(in_maps)
    total = np.float64(0.0)
    for i in range(N_CORES):
        total += np.float64(res[i]["out"][0, 0])
    return np.asarray(total, dtype=np.float32)

